# revision 28
# baseline (speedup 1.0000x reference)
"""Distributed Bass kernel for GQA causal attention (B=2, S=2048, H=2048,
NH=16, NKV=4, HD=128) on 8 TRN2 NeuronCores.

Sharding: core c (0..7) handles batch b = c//4 and kv-group g = c%4
(4 query heads + 1 kv head, GQA groups kept intact).  wq/wk/wv are
column-sharded, wo row-sharded; each core emits a partial output
[H, S] (transposed) and the host sums the 4 group-partials per batch.

v2 design (vs v1): heads-concatenated attention + engine spreading.
  - GQA lets all 4 q-heads share each kv head, so scores/attnV/rowsum
    stream all 4 heads as one wide free dim ([128, 4, 256] tiles):
    3x fewer PE instructions in attention, longer streams per weight
    load, and exactly 8 PSUM banks: scores 2x2 + ot 2 + rs 2.
  - causal mask folded into the scores accumulation group as a
    rank-structured matmul (ut.T @ vm = -1e30*max(0, kj-qi)), freeing
    the DVE of all mask adds.
  - q-blocks of 256 (8 of them); wo projection runs in 4 chunks of
    512 qi (after qb 1,3,5,7), reusing the freed ot/rs PSUM banks.
    Chunk-boundary latency (recip on ACT -> norm on DVE) is hidden by
    2 lookahead score tiles of the next qb + first-2-e-pairs of wo
    computed on the (long-ready) first half of the OT chunk.
  - ot PSUM is released early: DVE copies ot->SBUF right after the
    last attnV, then the 1/rowsum scale happens SBUF-side, so wo's
    PSUM slots are free before the reciprocal finishes.
  - element-wise work is spread over three engines: ACT does the
    phase-1 PSUM drains + exp + recip, DVE does rope muls / norm /
    output casts, Pool (gpsimd) does the SBUF-only rope adds and the
    deferred block-3 rope chunks.
  - phase 1 (QKV projections + RoPE) keeps the v1 software pipeline:
    per-t interleaved critical-path DMAs, per-block PSUM skew, rope
    chunks interleaved into the next block's matmul groups.
"""

import math
import os
import sys

import ml_dtypes
import numpy as np

sys.path.insert(0, "/opt/trn_rl_repo")

import concourse.bass as bass
import concourse.mybir as mybir
import concourse.tile as tile
from concourse.bass_utils import run_bass_kernel_spmd

B, S, H = 2, 2048, 2048
NH, NKV, HD = 16, 4, 128
NREP = NH // NKV
NCORES = 8
GH = 4                # q-heads per core (one kv group)
P = 128
SB = 512              # phase-1 s-block width
NB = S // SB          # 4 s-blocks
NT = S // P           # 16 partition tiles along s / h / e
QB = 256              # attention q-block width
NQB = S // QB         # 8 q-blocks
SCALE = 1.0 / math.sqrt(HD)
F32 = mybir.dt.float32
BF16 = mybir.dt.bfloat16
MMDT = BF16
NPMM = ml_dtypes.bfloat16
OUTDT = BF16
NPOUT = ml_dtypes.bfloat16
TRIGDT = BF16
NPTRIG = ml_dtypes.bfloat16
EXP = mybir.ActivationFunctionType.Exp
LN = mybir.ActivationFunctionType.Ln


def _consts():
    npdt = NPMM
    # rotate_half as matmul: rot = RT.T @ q  (RT is the lhsT)
    RT = np.zeros((P, P), npdt)
    idx = np.arange(64)
    RT[idx + 64, idx] = -1.0
    RT[idx, idx + 64] = 1.0
    ident = np.eye(P, dtype=npdt)
    ones_f = np.ones((P, P), npdt)
    # causal mask as a rank-structured matmul: (ut.T @ vm)[kj, qi]
    #   = -1e30 * #{t : qi < t <= kj} = -1e30 * max(0, kj - qi).
    # Matmul moving operands must be one contiguous free dim, so the
    # masks are materialized at full attention-tile width [t, GH*QB]:
    #   vmA (kj tile nkj-2): per head [tri | zeros]
    #   vmB (kj tile nkj-1): per head [all -1e30 | tri]
    ut = np.triu(np.ones((P, P), np.float32))                   # [t, kj]
    vm = np.tril(np.full((P, P), -1e30, np.float32), -1)        # [t, qi]
    zero = np.zeros((P, P), np.float32)
    neg = np.full((P, P), -1e30, np.float32)
    vmA = np.concatenate([vm, zero], axis=1)                    # [t, QB]
    vmB = np.concatenate([neg, vm], axis=1)                     # [t, QB]
    vmA4 = np.tile(vmA[:, None, :], (1, GH, 1)).reshape(P, GH * QB)
    vmB4 = np.tile(vmB[:, None, :], (1, GH, 1)).reshape(P, GH * QB)
    return (RT, ident, ones_f, ut.astype(npdt),
            vmA4.astype(npdt), vmB4.astype(npdt))


def build_nc():
    nc = bass.Bass()

    xT_d = nc.declare_dram_parameter("xT", [H, S], MMDT, isOutput=False)
    # weights come host-pre-shuffled to the SBUF image layout
    # [p, t, e] = w[t*128+p, e] so each partition's data is one long
    # contiguous DRAM run (4KB DMA packets instead of 1KB/256B rows)
    wq_d = nc.declare_dram_parameter("wq", [P, NT * GH * HD], MMDT,
                                     isOutput=False)
    wk_d = nc.declare_dram_parameter("wk", [P, NT * HD], MMDT,
                                     isOutput=False)
    wv_d = nc.declare_dram_parameter("wv", [P, NT * HD], MMDT,
                                     isOutput=False)
    wo_d = nc.declare_dram_parameter("wo", [GH * HD, H], MMDT, isOutput=False)
    cosT_d = nc.declare_dram_parameter("cosT", [HD, S], TRIGDT,
                                       isOutput=False)
    sinT_d = nc.declare_dram_parameter("sinT", [HD, S], TRIGDT,
                                       isOutput=False)
    out_d = nc.declare_dram_parameter("out", [H, S], OUTDT, isOutput=True)

    RT_np, ident_np, ones_f_np, ut_np, vmA_np, vmB_np = _consts()
    RT_d = nc.inline_tensor(RT_np, "rot_t")
    ident_d = nc.inline_tensor(ident_np, "ident")
    ones_f_d = nc.inline_tensor(ones_f_np, "ones_f")
    ut_d = nc.inline_tensor(ut_np, "ut_mask")
    vmA_d = nc.inline_tensor(vmA_np, "vmA_mask")
    vmB_d = nc.inline_tensor(vmB_np, "vmB_mask")

    with tile.TileContext(nc) as tc, \
         tc.tile_pool(name="persist", bufs=1) as persist:
        rt_sb = persist.tile([P, P], MMDT, tag="rt")
        ident_sb = persist.tile([P, P], MMDT, tag="ident")
        ones_sb = persist.tile([P, P], MMDT, tag="ones_f")
        ut_sb = persist.tile([P, P], MMDT, tag="ut")
        vmA_sb = persist.tile([P, GH * QB], MMDT, tag="vmA")
        vmB_sb = persist.tile([P, GH * QB], MMDT, tag="vmB")
        cos_sb = persist.tile([P, S], TRIGDT, tag="cos")
        sin_sb = persist.tile([P, S], TRIGDT, tag="sin")

        # resident weights (each element used once per s-block)
        wq_sb = persist.tile([P, NT, GH * HD], MMDT, tag="wq")
        wk_sb = persist.tile([P, NT, HD], MMDT, tag="wk")
        wv_sb = persist.tile([P, NT, HD], MMDT, tag="wv")

        # roped Q, qb-major so each q-block's 4 heads are one
        # contiguous 1024-wide run (matmul moving operands must be a
        # single free dim): [d, qb, h, qi]; K^T [d, s];
        # V in attnV-lhsT layout [s_local, kj_tile, d]
        QR_flat = persist.tile([P, NQB * GH * QB], MMDT, tag="qr_all")
        QR4 = QR_flat.rearrange("p (a h w) -> p a h w", a=NQB, h=GH)
        KR_all = persist.tile([P, S], MMDT, tag="kr_all")
        VV_all = persist.tile([P, NT, P], MMDT, tag="vv_all")
        VT = [persist.tile([P, SB], MMDT, tag=f"vt{b}", name=f"vt{b}")
              for b in range(NB)]

        # normalized attention outputs, chunked for the wo projection:
        # [d, h, 512 qi] per chunk, double-buffered
        OTc = [persist.tile([P, GH, 2 * QB], MMDT, tag=f"otc{i}",
                            name=f"otc{i}") for i in range(2)]

        # staging for block 3's RoPE, finished inside phase 2
        raw3 = [persist.tile([P, SB], MMDT, tag=f"raw3_{i}",
                             name=f"raw3_{i}") for i in range(5)]
        rot3 = [persist.tile([P, SB], MMDT, tag=f"rot3_{i}",
                             name=f"rot3_{i}") for i in range(5)]

        # ---------------- Phase 1: projections + RoPE ----------------
        # Software-pipelined as in v1: block b's PSUM tiles drain to
        # SBUF (ACT copies) right after its matmuls; rope chunks are
        # interleaved into block b+1's matmul stream.
        # x tiles load as full DRAM rows (4KB contiguous runs -> 4KB
        # DMA packets) and stay resident for all four s-blocks; the
        # weights come host-shuffled so their DMAs are also one long
        # run per partition.  This quadruples early DMA throughput --
        # the v2/v3 traces showed the PE starving on block-0 loads.
        xT_r = xT_d.rearrange("(a p) s -> p a s", p=P)
        wq_sb_f = wq_sb.rearrange("p a b -> p (a b)")
        wk_sb_f = wk_sb.rearrange("p a b -> p (a b)")
        wv_sb_f = wv_sb.rearrange("p a b -> p (a b)")

        with (
            tc.tile_pool(name="p1w", bufs=3) as p1w,
            tc.tile_pool(name="p1ps", bufs=1, space="PSUM") as p1ps,
            tc.tile_pool(name="rotps", bufs=2, space="PSUM") as rotps,
        ):
            xfull = persist.tile([P, NT, S], MMDT, tag="xfull")

            # critical path: x t=0 and wq t=0 split across two queues
            # each, then per-t singles so tiles land progressively on
            # rotating queues (one monolithic DMA would gate the first
            # matmul on the whole transfer)
            EW = GH * HD
            for hp in range(2):
                nc.sync.dma_start(out=xfull[hp * 64:(hp + 1) * 64, 0, :],
                                  in_=xT_r[hp * 64:(hp + 1) * 64, 0, :])
                nc.sync.dma_start(out=wq_sb_f[hp * 64:(hp + 1) * 64, 0:EW],
                                  in_=wq_d[hp * 64:(hp + 1) * 64, 0:EW])
            for t in range(1, 4):
                nc.sync.dma_start(out=xfull[:, t, :], in_=xT_r[:, t, :])
                nc.sync.dma_start(out=wq_sb_f[:, t * EW:(t + 1) * EW],
                                  in_=wq_d[:, t * EW:(t + 1) * EW])
            nc.sync.dma_start(out=wk_sb_f, in_=wk_d[:])
            nc.sync.dma_start(out=wv_sb_f, in_=wv_d[:])
            for t in range(4, NT):
                nc.sync.dma_start(out=xfull[:, t, :], in_=xT_r[:, t, :])
                if t % 4 == 0:
                    nc.sync.dma_start(
                        out=wq_sb_f[:, t * EW:(t + 4) * EW],
                        in_=wq_d[:, t * EW:(t + 4) * EW])
            nc.sync.dma_start(out=rt_sb, in_=RT_d[:])
            nc.sync.dma_start(out=ident_sb, in_=ident_d[:])
            nc.sync.dma_start(out=ones_sb, in_=ones_f_d[:])
            nc.sync.dma_start(out=ut_sb, in_=ut_d[:])
            nc.sync.dma_start(out=vmA_sb, in_=vmA_d[:])
            nc.sync.dma_start(out=vmB_sb, in_=vmB_d[:])

            def _mm_i(ps, i, t, sb, st, sp):
                if i < GH:
                    w = wq_sb[:, t, i * HD:(i + 1) * HD]
                elif i == 4:
                    w = wk_sb[:, t, :]
                else:
                    w = wv_sb[:, t, :]
                nc.tensor.matmul(ps[i], w,
                                 xfull[:, t, sb * SB:(sb + 1) * SB],
                                 start=st, stop=sp)

            def _sched(delays):
                """per-ps emission schedule: ps_i's NT matmuls spread
                evenly over groups delays[i]..NT-1."""
                out = [[[] for _ in range(6)] for _ in range(NT)]
                for i, d in enumerate(delays):
                    gs = list(range(d, NT))
                    n, k = NT, len(gs)
                    nxt = 0
                    for gi, g in enumerate(gs):
                        take = (n * (gi + 1)) // k - (n * gi) // k
                        for _ in range(take):
                            out[g][i].append(nxt)
                            nxt += 1
                return out

            def _make_rope_chunks(sb, raws, vt):
                ssl = slice(sb * SB, (sb + 1) * SB)
                chunks = []
                for i in range(5):
                    def _c(i=i, raw_r=raws[i], ssl=ssl, sb=sb):
                        rot = rotps.tile([P, SB], F32, tag="rv",
                                         name=f"rot{sb}_{i}")
                        nc.tensor.matmul(rot, rt_sb, raw_r)
                        t1 = p1w.tile([P, SB], F32, tag="t1",
                                      name=f"t1_{sb}_{i}")
                        nc.vector.tensor_mul(t1, raw_r, cos_sb[:, ssl])
                        t2 = p1w.tile([P, SB], F32, tag="t2",
                                      name=f"t2_{sb}_{i}")
                        nc.vector.tensor_mul(t2, rot, sin_sb[:, ssl])
                        if i < GH:
                            # [2 qbs, 256] view of this 512-wide s-block
                            dst = QR4[:, 2 * sb:2 * sb + 2, i, :]
                            t1v = t1.rearrange("p (a w) -> p a w", a=2)
                            t2v = t2.rearrange("p (a w) -> p a w", a=2)
                        else:
                            dst = KR_all[:, ssl]
                            t1v, t2v = t1, t2
                        # SBUF-only add on the Pool engine
                        nc.gpsimd.tensor_add(dst, t1v, t2v)
                    chunks.append(_c)
                for tt in range(SB // P):
                    def _v(tt=tt, vt=vt, sb=sb):
                        vps = rotps.tile([P, P], MMDT, tag="rv",
                                         name=f"vtr{sb}_{tt}")
                        nc.tensor.transpose(
                            vps, vt[:, tt * P:(tt + 1) * P], ident_sb)
                        nc.scalar.copy(
                            VV_all[:, sb * (SB // P) + tt, :], vps)
                    chunks.append(_v)
                return chunks

            pending_rope = []
            for sb in range(NB):
                if sb == 0:
                    nc.sync.dma_start(out=cos_sb, in_=cosT_d[:])
                    nc.sync.dma_start(out=sin_sb, in_=sinT_d[:])
                ps = [p1ps.tile([P, SB], F32, tag=f"ps{i}", name=f"ps{i}")
                      for i in range(6)]
                delays = [0] * 6 if sb == 0 else [2, 3, 3, 3, 4, 4]
                sched = _sched(delays)
                started = [False] * 6
                left = [NT] * 6
                chunks = list(pending_rope)
                for g in range(NT):
                    for i in range(6):
                        for t in sched[g][i]:
                            left[i] -= 1
                            _mm_i(ps, i, t, sb,
                                  st=not started[i], sp=left[i] == 0)
                            started[i] = True
                    ci = g - 3
                    if 0 <= ci < len(chunks):
                        chunks[ci]()
                # drain PSUM -> SBUF on ACT (frees banks for the next
                # block without loading the DVE)
                if sb < NB - 1:
                    raws = [p1w.tile([P, SB], MMDT, tag="raw", bufs=10,
                                     name=f"raw{sb}_{i}") for i in range(5)]
                else:
                    raws = raw3
                for i in range(5):
                    nc.scalar.copy(raws[i], ps[i])
                nc.scalar.copy(VT[sb], ps[5])
                pending_rope = _make_rope_chunks(sb, raws, VT[sb])

            # block 3: rot matmuls + stage rot to SBUF inside the pool
            # scope; cos/sin combine is deferred into phase 2
            for i in range(5):
                rot = rotps.tile([P, SB], F32, tag="rv", name=f"rot3_{i}")
                nc.tensor.matmul(rot, rt_sb, raw3[i])
                nc.vector.tensor_copy(rot3[i], rot)
            for tt in range(SB // P):
                vps = rotps.tile([P, P], MMDT, tag="rv", name=f"vtr3_{tt}")
                nc.tensor.transpose(
                    vps, VT[3][:, tt * P:(tt + 1) * P], ident_sb)
                nc.scalar.copy(VV_all[:, 3 * (SB // P) + tt, :], vps)

        # -------- Phase 2+3: attention (qb outer) + wo projection --------
        with (
            tc.tile_pool(name="pp", bufs=7) as pp,
            tc.tile_pool(name="recp", bufs=2) as recp,
            tc.tile_pool(name="otup", bufs=2) as otup,
            tc.tile_pool(name="oep", bufs=3) as oep,
            tc.tile_pool(name="stps", bufs=2, space="PSUM") as stps,
            tc.tile_pool(name="accps", bufs=2, space="PSUM") as accps,
        ):
            # wo shares wq_sb's slot (dead after phase 1)
            wo_sb = wq_sb.rearrange("p a b -> p (a b)").rearrange(
                "p (g e) -> p g e", g=GH)
            for hh in range(GH):
                nc.sync.dma_start(out=wo_sb[:, hh, :],
                                  in_=wo_d[hh * P:(hh + 1) * P, :])

            def _rope3_chunk(i):
                # SBUF-only: runs on the Pool engine
                ssl = slice(3 * SB, 4 * SB)
                t1 = recp.tile([P, SB], F32, tag="r3a", name=f"r3t1_{i}")
                nc.gpsimd.tensor_mul(t1, raw3[i], cos_sb[:, ssl])
                t2 = recp.tile([P, SB], F32, tag="r3b", name=f"r3t2_{i}")
                nc.gpsimd.tensor_mul(t2, rot3[i], sin_sb[:, ssl])
                if i < GH:
                    dst = QR4[:, 6:8, i, :]
                    t1v = t1.rearrange("p (a w) -> p a w", a=2)
                    t2v = t2.rearrange("p (a w) -> p a w", a=2)
                else:
                    dst = KR_all[:, ssl]
                    t1v, t2v = t1, t2
                nc.gpsimd.tensor_add(dst, t1v, t2v)

            # K of block 3 first (needed earliest, by qb6's scores)
            rope3_left = [4, 0, 1, 2, 3]

            HW_ = GH * QB // 2  # 512: matmul moving operands cap at 512

            def sc_exp(qb, kj, nkj):
                """scores (+mask) for one kj tile, all heads; exp to P.
                Full-width [128, GH*QB] tiles; matmuls emitted in two
                512-wide halves (ISA caps the moving operand at 512
                elements).  The two diagonal kj tiles get their causal
                mask added in-group via extra matmuls (ut.T @ vmA/vmB)."""
                stf = stps.tile([P, GH * QB], F32, tag="st",
                                name=f"st{qb}_{kj}")
                diag1, diag2 = kj == nkj - 2, kj == nkj - 1
                kr = KR_all[:, kj * P:(kj + 1) * P]
                q0 = qb * GH * QB
                for hf in range(2):
                    sl = slice(hf * HW_, (hf + 1) * HW_)
                    nc.tensor.matmul(stf[:, sl], kr,
                                     QR_flat[:, q0 + hf * HW_:
                                             q0 + (hf + 1) * HW_],
                                     start=True,
                                     stop=not (diag1 or diag2),
                                     skip_group_check=True)
                if diag1 or diag2:
                    vmask = vmA_sb if diag1 else vmB_sb
                    for hf in range(2):
                        sl = slice(hf * HW_, (hf + 1) * HW_)
                        nc.tensor.matmul(stf[:, sl], ut_sb, vmask[:, sl],
                                         start=False, stop=True,
                                         skip_group_check=True)
                p = pp.tile([P, GH * QB], MMDT, tag="p", name=f"p{qb}_{kj}")
                nc.scalar.activation(p, stf, EXP, scale=SCALE)
                return p

            def av(kj, p, ot, rs, nkj):
                first, last = kj == 0, kj == nkj - 1
                for hf in range(2):
                    sl = slice(hf * HW_, (hf + 1) * HW_)
                    nc.tensor.matmul(rs[:, sl], ones_sb, p[:, sl],
                                     start=first, stop=last,
                                     skip_group_check=True)
                for hf in range(2):
                    sl = slice(hf * HW_, (hf + 1) * HW_)
                    nc.tensor.matmul(ot[:, sl], VV_all[:, kj, :], p[:, sl],
                                     start=first, stop=last,
                                     skip_group_check=True)

            CW = 2 * QB  # wo chunk width (512 qi)

            def wo_chunk(ch, otc, last_chunk, head_pairs):
                """wo projection for qi chunk ch (CW wide).
                head_pairs: e-pairs already emitted in half-qi mode."""
                for ep in range(NT // 2):
                    if ep < head_pairs:
                        continue
                    # last e-pair borrows a scores slot so the next
                    # qb's ot/rs allocation doesn't wait on its cast
                    pool = stps if ep == NT // 2 - 1 else accps
                    tag = "st" if ep == NT // 2 - 1 else "acc"
                    o_s = pool.tile([P, GH * QB], F32, tag=tag,
                                    name=f"wo{ch}_{ep}")
                    for hf in range(2):
                        e = 2 * ep + hf
                        for h in range(GH):
                            nc.tensor.matmul(
                                o_s[:, hf * CW:(hf + 1) * CW],
                                wo_sb[:, h, e * P:(e + 1) * P],
                                otc[:, h, :],
                                start=h == 0, stop=h == GH - 1,
                                skip_group_check=True)
                    _wo_drain(ch, ep, o_s, last_chunk)

            def _wo_drain(ch, ep, o_s, last_chunk):
                csl = slice(ch * CW, (ch + 1) * CW)
                oe = oep.tile([P, 2 * CW], OUTDT, tag="oe",
                              name=f"oe{ch}_{ep}")
                if last_chunk:
                    # tail: per-half casts so the first DMA issues while
                    # the second half is still casting, and issue on the
                    # (idle) gpsimd SWDGE path to bypass the Sync-engine
                    # issue queue
                    for hf in range(2):
                        e = 2 * ep + hf
                        esl = slice(hf * CW, (hf + 1) * CW)
                        nc.vector.tensor_copy(oe[:, esl], o_s[:, esl])
                        nc.gpsimd.dma_start(out=out_d[e * P:(e + 1) * P, csl],
                                            in_=oe[:, esl])
                else:
                    nc.vector.tensor_copy(oe, o_s)
                    for hf in range(2):
                        e = 2 * ep + hf
                        esl = slice(hf * CW, (hf + 1) * CW)
                        nc.sync.dma_start(out=out_d[e * P:(e + 1) * P, csl],
                                          in_=oe[:, esl])

            pre = []
            for qb in range(NQB):
                nkj = 2 * (qb + 1)
                otf = accps.tile([P, GH * QB], F32, tag="acc",
                                 name=f"ot{qb}")
                rsf = accps.tile([P, GH * QB], F32, tag="acc",
                                 name=f"rs{qb}")

                if 1 <= qb <= 5 and rope3_left:
                    _rope3_chunk(rope3_left.pop(0))

                tiles = {}
                for kj, pq in enumerate(pre):
                    tiles[kj] = pq
                pre = []
                ks, avd = len(tiles), 0
                while avd < nkj:
                    if ks < nkj and ks - avd < 3:
                        tiles[ks] = sc_exp(qb, ks, nkj)
                        ks += 1
                    else:
                        av(avd, tiles.pop(avd), otf, rsf, nkj)
                        avd += 1

                # reciprocal on ACT; early PSUM release: copy ot->SBUF
                # on DVE (no recip dependency), scale later
                lnr = recp.tile([P, GH * QB], F32, tag="lnr",
                                name=f"lnr{qb}")
                nc.scalar.activation(lnr, rsf, LN)
                otu = otup.tile([P, GH * QB], F32, tag="otu",
                                name=f"otu{qb}")
                nc.vector.tensor_copy(otu, otf)
                rec = recp.tile([P, GH * QB], F32, tag="rec",
                                name=f"rec{qb}")
                nc.scalar.activation(rec, lnr, EXP, scale=-1.0)
                otc = OTc[(qb // 2) % 2]
                half = qb % 2
                dst = otc[:, :, half * QB:(half + 1) * QB]
                nc.vector.tensor_mul(
                    dst, otu.rearrange("p (h w) -> p h w", h=GH),
                    rec.rearrange("p (h w) -> p h w", h=GH))

                if qb % 2 == 1:
                    ch = qb // 2
                    last_chunk = qb == NQB - 1
                    # lookahead scores of the next qb keep PE busy and
                    # feed ACT during the wo chunk
                    if not last_chunk:
                        nn = 2 * (qb + 2)
                        pre = [sc_exp(qb + 1, 0, nn), sc_exp(qb + 1, 1, nn)]
                    # first 2 e-pairs: compute the first-half (even qb)
                    # columns now -- OTc half 0 was normalized a whole
                    # qb ago, so these don't wait on this qb's recip
                    head = 2
                    o_head = []
                    for ep in range(head):
                        o_s = accps.tile([P, GH * QB], F32, tag="acc",
                                         name=f"woh{ch}_{ep}")
                        for hf in range(2):
                            for h in range(GH):
                                nc.tensor.matmul(
                                    o_s[:, hf * CW:hf * CW + QB],
                                    wo_sb[:, h, (2 * ep + hf) * P:
                                          (2 * ep + hf + 1) * P],
                                    otc[:, h, 0:QB],
                                    start=h == 0, stop=h == GH - 1,
                                    skip_group_check=True)
                        o_head.append(o_s)
                    if not last_chunk:
                        nn = 2 * (qb + 2)
                        pre.append(sc_exp(qb + 1, 2, nn))
                    for ep in range(head):
                        o_s = o_head[ep]
                        for hf in range(2):
                            for h in range(GH):
                                nc.tensor.matmul(
                                    o_s[:, hf * CW + QB:(hf + 1) * CW],
                                    wo_sb[:, h, (2 * ep + hf) * P:
                                          (2 * ep + hf + 1) * P],
                                    otc[:, h, QB:2 * QB],
                                    start=h == 0, stop=h == GH - 1,
                                    skip_group_check=True)
                        _wo_drain(ch, ep, o_s, last_chunk)
                    wo_chunk(ch, otc, last_chunk, head)

    _hoist_matmul_waits(nc)
    return nc


_HOIST_OPS = {"Matmult", "DMACopy"}


def _hoist_matmul_waits(nc):
    """Self-loading matmuls (and direct2d DMAs) only support ONE
    sync-wait -- walrus puts all waits on one ISA struct.  Hoist extra
    waits onto standalone single-wait EventSemaphores inserted right
    before the offending instruction on the same engine."""
    n_fixed = 0
    for fn in nc.m.functions:
        for blk in fn.blocks:
            out = []
            for inst in blk.instructions:
                si = inst.sync_info
                if (inst.opcode != "EventSemaphore" and si is not None
                        and si.on_wait is not None and len(si.on_wait) > 1):
                    waits = list(si.on_wait)
                    for wi, w in enumerate(waits[:-1]):
                        out.append(mybir.InstEventSemaphore(
                            name=f"hoistw_{inst.name}_{wi}", ins=[], outs=[],
                            sync_info=mybir.SyncInfo(on_wait=[w],
                                                     on_update=[]),
                            engine=inst.engine))
                    inst.sync_info = mybir.SyncInfo(
                        on_wait=[waits[-1]],
                        on_update=list(si.on_update or []))
                    n_fixed += 1
                out.append(inst)
            blk.instructions = out
    return n_fixed


def _shuf(w):
    """[H, E] -> [P, NT*E] SBUF-image layout: [p, t*E+e] = w[t*128+p, e]"""
    E = w.shape[1]
    return np.ascontiguousarray(
        w.reshape(NT, P, E).transpose(1, 0, 2).reshape(P, NT * E))


def make_in_maps(x, cos, sin, wq, wk, wv, wo):
    cosT = np.ascontiguousarray(cos.T.astype(NPTRIG))
    sinT = np.ascontiguousarray(sin.T.astype(NPTRIG))
    xT = [np.ascontiguousarray(x[b].T.astype(NPMM)) for b in range(B)]
    wq, wk, wv, wo = (a.astype(NPMM) for a in (wq, wk, wv, wo))
    in_maps = []
    for c in range(NCORES):
        b, g = divmod(c, NKV)
        in_maps.append({
            "xT": xT[b],
            "wq": _shuf(wq[:, g * GH * HD:(g + 1) * GH * HD]),
            "wk": _shuf(wk[:, g * HD:(g + 1) * HD]),
            "wv": _shuf(wv[:, g * HD:(g + 1) * HD]),
            "wo": np.ascontiguousarray(wo[g * GH * HD:(g + 1) * GH * HD, :]),
            "cosT": cosT,
            "sinT": sinT,
        })
    return in_maps


_NC_CACHE = {}


def _get_nc():
    if "nc" not in _NC_CACHE:
        _NC_CACHE["nc"] = build_nc()
    return _NC_CACHE["nc"]


N_WARMUP = int(os.environ.get("BASS_WARMUP", "2"))


def run(x, cos, sin, wq, wk, wv, wo, **spmd_kwargs):
    nc = _get_nc()
    in_maps = make_in_maps(x, cos, sin, wq, wk, wv, wo)
    # Warm the device (DVFS/p-state ramps, DMA rings, NEFF residency)
    for _ in range(N_WARMUP):
        try:
            from concourse import bass2jax
            bass2jax.run_bass_via_pjrt(nc, in_maps, n_cores=NCORES)
        except Exception:
            break
    res = run_bass_kernel_spmd(nc, in_maps, core_ids=list(range(NCORES)),
                               **spmd_kwargs)
    outs = [np.asarray(res.results[c]["out"]).astype(np.float32)
            for c in range(NCORES)]
    full = np.empty((B, S, H), np.float32)
    for b in range(B):
        acc = outs[4 * b]
        for g in range(1, NKV):
            acc = acc + outs[4 * b + g]
        full[b] = acc.T
    return full, res


def kernel(**inputs):
    out, _ = run(**inputs)
    return out


if __name__ == "__main__":
    import tempfile
    from concourse.bass_utils import compile_bir_kernel

    nc = build_nc()
    print("graph built OK")
    if os.environ.get("COMPILE_CHECK", "1") == "1":
        td = tempfile.mkdtemp(prefix="bass_compile_")
        neff = compile_bir_kernel(nc.to_json_bytes(), td, "kernel.neff")
        print(f"compiled OK: {neff}")
